# revision 1
# baseline (speedup 1.0000x reference)
"""AttnBlock (GroupNorm + single-head self-attention + residual) on 8 TRN2 cores.

Strategy: data-parallel over batch (16 images -> 2 per core). Each core runs an
identical Bass/Tile program on its slice; no collectives. All heavy matmuls run
in float32r (full-rate fp32 PE mode); GroupNorm statistics and the final
residual combine stay in plain fp32.

Per-batch dataflow on one core (C=512 channels, N=H*W=1024 tokens):
  x    [C, N]  channel-major (native layout of the input)
  h    = groupnorm(x)                      (stats via bn_stats + tiny matmuls)
  qT   [o, n] = wqT.T @ h    (4 c-tiles accumulated in PSUM)
  kT   [o, m] = wkT.T @ h
  v    [m, c] = h.T @ wvT    (token-major, produced directly by swapping
                              matmul operand roles -- no transposes anywhere)
  sT   [m, n] = kT.T @ qT    -> exp(sT / sqrt(C)) on ACT (no max-subtraction:
                              scores are O(1) by construction)
  den  [1, n] = ones.T @ exp (PSUM-accumulated over the 8 m-tiles)
  ctxu [c, n] = v.T @ exp    (unnormalized)
  yu   [p, n] = wpT.T @ ctxu
  out  = x + yu * bcast(1/den) + bp        (softmax normalization is deferred
                                            through the two linear stages)
"""

import numpy as np

B, C, HW = 16, 512, 1024
H = W = 32
NCORES = 8
BPC = B // NCORES
GROUPS = 32
GSIZE = C // GROUPS  # 16
EPS = 1e-5

_CACHE = {}


def _build_nc():
    import concourse.bacc as bacc
    import concourse.tile as tile
    from concourse import mybir

    R = mybir.dt.float32r
    F = mybir.dt.float32
    A = mybir.AluOpType
    AF = mybir.ActivationFunctionType

    nc = bacc.Bacc("TRN2", target_bir_lowering=False, debug=False)

    x = nc.declare_dram_parameter("x", [BPC, C, HW], F, isOutput=False)
    wq = nc.declare_dram_parameter("wq", [C, C], R, isOutput=False)  # [c, o]
    wk = nc.declare_dram_parameter("wk", [C, C], R, isOutput=False)
    wv = nc.declare_dram_parameter("wv", [C, C], R, isOutput=False)
    wp = nc.declare_dram_parameter("wp", [C, C], R, isOutput=False)
    vecs = nc.declare_dram_parameter("vecs", [128, 4, 5], F, isOutput=False)
    bvb = nc.declare_dram_parameter("bvb", [128, 512], F, isOutput=False)
    gmask = nc.declare_dram_parameter("gmask", [128, 8], F, isOutput=False)
    gmaskT = nc.declare_dram_parameter("gmaskT", [8, 128], F, isOutput=False)
    ones_col = nc.declare_dram_parameter("ones_col", [128, 1], R, isOutput=False)
    ones_row = nc.declare_dram_parameter("ones_row", [1, 128], R, isOutput=False)
    y = nc.declare_dram_parameter("y", [BPC, C, HW], F, isOutput=True)

    with tile.TileContext(nc) as tc:
        import contextlib

        ctx = contextlib.ExitStack()
        with ctx:
            wpool = ctx.enter_context(tc.tile_pool(name="w", bufs=1))
            cpool = ctx.enter_context(tc.tile_pool(name="c", bufs=1))
            xpool = ctx.enter_context(tc.tile_pool(name="x", bufs=2))
            hpool = ctx.enter_context(tc.tile_pool(name="h", bufs=2))
            qpool = ctx.enter_context(tc.tile_pool(name="q", bufs=1))
            kpool = ctx.enter_context(tc.tile_pool(name="k", bufs=1))
            vpool = ctx.enter_context(tc.tile_pool(name="v", bufs=1))
            epool = ctx.enter_context(tc.tile_pool(name="e", bufs=1))
            spool = ctx.enter_context(tc.tile_pool(name="s", bufs=2))
            rpool = ctx.enter_context(tc.tile_pool(name="r", bufs=1))
            opool = ctx.enter_context(tc.tile_pool(name="o", bufs=3))
            mpool = ctx.enter_context(tc.tile_pool(name="mp", bufs=6, space="PSUM"))
            gpool = ctx.enter_context(tc.tile_pool(name="gp", bufs=2, space="PSUM"))

            # ---- persistent loads -------------------------------------------
            # batch-0 x tiles first: the whole pipeline's critical path starts
            # with groupnorm stats, so get those bytes moving before weights.
            xts = []
            for b in range(BPC):
                xt_b = xpool.tile([128, 4, HW], F, tag="x", name=f"xt{b}")
                xts.append(xt_b)
            xsrc = [x.ap()[b].rearrange("(i p) n -> p i n", p=128) for b in range(BPC)]
            from concourse.tile import add_dep_helper

            # DMA order = HBM-bandwidth priority order. Batch-0 x gates the
            # whole pipeline (groupnorm stats), so it goes first with nothing
            # competing; each later bulk transfer is chained behind the
            # previous one (sync deps) in first-use order so early consumers
            # are never starved by bytes that aren't needed until later.
            x0_dmas = []
            for i in range(4):
                for s in range(2):
                    d = nc.sync.dma_start(out=xts[0][:, i, s * 512 : (s + 1) * 512],
                                          in_=xsrc[0][:, i, s * 512 : (s + 1) * 512])
                    x0_dmas.append(d)
            gmask_t = cpool.tile([128, 8], F, tag="gmask")
            nc.sync.dma_start(out=gmask_t, in_=gmask.ap())
            gmaskT_t = cpool.tile([8, 128], F, tag="gmaskT")
            nc.sync.dma_start(out=gmaskT_t, in_=gmaskT.ap())
            vecs_t = cpool.tile([128, 4, 5], F, tag="vecs")
            nc.sync.dma_start(out=vecs_t, in_=vecs.ap())
            bvb_t = cpool.tile([128, 512], F, tag="bvb")
            nc.sync.dma_start(out=bvb_t, in_=bvb.ap())
            ones_col_t = cpool.tile([128, 1], R, tag="ones_col")
            nc.sync.dma_start(out=ones_col_t, in_=ones_col.ap())
            ones_row_t = cpool.tile([1, 128], R, tag="ones_row")
            nc.sync.dma_start(out=ones_row_t, in_=ones_row.ap())
            eps8 = cpool.tile([8, 1], F, tag="eps8")
            nc.vector.memset(eps8, EPS)

            # PE warmup: the tensor engine sits idle until groupnorm stats
            # arrive (~13us) and would start HAM-throttled at 1.2 GHz. A chain
            # of dummy fp32 matmuls on memset-zero tiles (no input deps) keeps
            # it busy and un-throttles the clock before the real work lands.
            wrm = cpool.tile([128, 128], F, tag="wrm")
            nc.vector.memset(wrm, 0.0)
            wps = mpool.tile([128, 512], F, tag="mm")

            def warmup(n):
                for j in range(n):
                    nc.tensor.matmul(wps[:, 0:128], wrm, wrm, start=(j == 0),
                                     stop=(j == n - 1))

            warmup(24)

            wq_t = wpool.tile([128, 4, C], R, tag="wq")
            wk_t = wpool.tile([128, 4, C], R, tag="wk")
            wv_t = wpool.tile([128, 4, C], R, tag="wv")
            wp_t = wpool.tile([128, 4, C], R, tag="wp")
            prev = x0_dmas[-1]
            bulk = [(wq_t, wq, None), (wk_t, wk, None), (wv_t, wv, None),
                    (None, None, 1), (wp_t, wp, None)]
            for t, src, xb in bulk:
                if xb is not None:
                    for i in range(4):
                        d = nc.sync.dma_start(out=xts[xb][:, i, :], in_=xsrc[xb][:, i, :])
                        add_dep_helper(d.ins, prev.ins, reason="dma bandwidth order")
                    prev = d
                else:
                    d = nc.sync.dma_start(
                        out=t, in_=src.ap().rearrange("(ct p) o -> p ct o", p=128))
                    add_dep_helper(d.ins, prev.ins, reason="dma bandwidth order")
                    prev = d

            # ---- groupnorm for both batches, pipelined per 128-channel tile.
            # Groups are 16 consecutive channels, so every group lives in
            # exactly one 128-channel tile: each tile's normalization chain is
            # independent and unblocks its projection matmuls early. Batch 1's
            # chain is emitted before batch 0's attention so it fills engine
            # idle time during batch 0's matmul phases.
            hts = []
            for b in range(BPC):
                xt = xts[b]
                ht = hpool.tile([128, 4, HW], R, tag="hctx", name=f"ht{b}")
                hts.append(ht)
                # batch 1 collects all four tiles' variances and runs ONE
                # batched Sqrt: its per-tile Sqrt ops otherwise land inside
                # batch 0's exp stream, and Sqrt/Exp conflict in the ACT
                # function table (1.3us reload per alternation).
                varga = spool.tile([8, 4], F, tag="varga")
                sda = spool.tile([8, 4], F, tag="sda")
                ggs = {}

                def finish(i, gg, b=b, xt=xt, ht=ht, sda=sda):
                    st2 = spool.tile([8, 2], F, tag=f"st2{i}")
                    with nc.allow_low_precision("groupnorm rstd"):
                        nc.vector.reciprocal(out=st2[:, 0:1], in_=sda[:, i : i + 1])
                    nc.vector.tensor_copy(out=st2[:, 1:2], in_=gg[:, 0:1])
                    bc = gpool.tile([128, 2], F, tag="gn")
                    nc.tensor.matmul(bc, gmaskT_t, st2, start=True, stop=True)
                    scale_c = spool.tile([128, 1], F, tag=f"scale{i}")
                    nc.vector.tensor_mul(out=scale_c, in0=bc[:, 0:1], in1=vecs_t[:, i, 0:1])
                    tmp = spool.tile([128, 1], F, tag=f"tmp{i}")
                    nc.vector.tensor_mul(out=tmp, in0=bc[:, 1:2], in1=scale_c)
                    shift_c = spool.tile([128, 1], F, tag=f"shift{i}")
                    nc.vector.tensor_sub(out=shift_c, in0=vecs_t[:, i, 1:2], in1=tmp)
                    if b == 0 and i < 3:
                        # keep the warmed-up PE fed while the next tile's
                        # groupnorm stats crunch through the vector engine
                        warmup(8 + 2 * i)
                    if b == 0:
                        # batch 0's normalize rides the idle ACT at startup so
                        # DVE can move straight to the next tile's stats;
                        # batch 1's stays on DVE to keep ACT clear for batch
                        # 0's exp stream (which gates the denominator chain).
                        nc.scalar.activation(out=ht[:, i, :], in_=xt[:, i, :],
                                             func=AF.Identity, bias=shift_c,
                                             scale=scale_c)
                    else:
                        nc.vector.tensor_scalar(
                            out=ht[:, i, :], in0=xt[:, i, :],
                            scalar1=scale_c, scalar2=shift_c, op0=A.mult, op1=A.add)

                for i in range(4):
                    xr = xt[:, i, :].rearrange("p (s d) -> p s d", d=512)
                    st6 = spool.tile([128, 2, 6], F, tag=f"st6{i}")
                    for s in range(2):
                        nc.vector.bn_stats(out=st6[:, s, :], in_=xr[:, s, :])
                    mv = spool.tile([128, 2], F, tag=f"mv{i}")
                    nc.vector.bn_aggr(out=mv, in_=st6)
                    # stats_i = per-channel (mean, E[x^2])
                    stats_i = spool.tile([128, 2], F, tag=f"stats{i}")
                    m2c = spool.tile([128, 1], F, tag=f"m2c{i}")
                    nc.vector.tensor_mul(out=m2c, in0=mv[:, 0:1], in1=mv[:, 0:1])
                    nc.vector.tensor_add(out=stats_i[:, 1:2], in0=mv[:, 1:2], in1=m2c)
                    nc.vector.tensor_copy(out=stats_i[:, 0:1], in_=mv[:, 0:1])
                    gps = gpool.tile([8, 2], F, tag="gn")
                    nc.tensor.matmul(gps, gmask_t, stats_i, start=True, stop=True)
                    # gg = (mean_g, Ex2_g) per group
                    gg = spool.tile([8, 2], F, tag=f"gg{i}")
                    ggs[i] = gg
                    nc.vector.tensor_scalar_mul(out=gg, in0=gps, scalar1=1.0 / GSIZE)
                    m2g = spool.tile([8, 1], F, tag=f"m2g{i}")
                    nc.vector.tensor_mul(out=m2g, in0=gg[:, 0:1], in1=gg[:, 0:1])
                    nc.vector.tensor_sub(out=varga[:, i : i + 1], in0=gg[:, 1:2],
                                         in1=m2g)
                    if b == 0:
                        nc.scalar.activation(out=sda[:, i : i + 1],
                                             in_=varga[:, i : i + 1],
                                             func=AF.Sqrt, bias=eps8, scale=1.0)
                        finish(i, gg)
                if b == 1:
                    nc.scalar.activation(out=sda, in_=varga, func=AF.Sqrt,
                                         bias=eps8, scale=1.0)
                    for i in range(4):
                        finish(i, ggs[i])

            for b in range(BPC):
                xt = xts[b]
                ht = hts[b]
                # ---- projections q, k (channel-major), v (token-major) ------
                # The two n-half accumulation groups are interleaved so that
                # consecutive matmuls share the same stationary operand
                # (measured ~12ns/MM cheaper weight path).
                qt = qpool.tile([128, 4, HW], R, tag="q")
                kt = kpool.tile([128, 4, HW], R, tag="k")
                for dst, w_t, bidx in ((qt, wq_t, 2), (kt, wk_t, 3)):
                    for ot in range(4):
                        pp2 = [mpool.tile([128, 512], F, tag="mm",
                                          name=f"pj{b}_{bidx}_{ot}_{nh}")
                               for nh in range(2)]
                        for ct in range(4):
                            for nh in range(2):
                                nc.tensor.matmul(
                                    pp2[nh],
                                    w_t[:, ct, ot * 128 : (ot + 1) * 128],
                                    ht[:, ct, nh * 512 : (nh + 1) * 512],
                                    start=(ct == 0), stop=(ct == 3))
                        for nh in range(2):
                            if dst is kt:
                                # split the psum evacuations across engines:
                                # q on DVE, k on ACT, so neither backlogs at
                                # the projections->scores boundary
                                nc.scalar.activation(
                                    out=dst[:, ot, nh * 512 : (nh + 1) * 512],
                                    in_=pp2[nh], func=AF.Identity,
                                    bias=vecs_t[:, ot, bidx : bidx + 1], scale=1.0)
                            else:
                                nc.vector.tensor_scalar_add(
                                    out=dst[:, ot, nh * 512 : (nh + 1) * 512],
                                    in0=pp2[nh],
                                    scalar1=vecs_t[:, ot, bidx : bidx + 1])
                vt = vpool.tile([128, 8, 512], R, tag="v")
                for mt in range(8):
                    ps = mpool.tile([128, 512], F, tag="mm")
                    for ct in range(4):
                        nc.tensor.matmul(
                            ps,
                            ht[:, ct, mt * 128 : (mt + 1) * 128],
                            wv_t[:, ct, :],
                            start=(ct == 0), stop=(ct == 3))
                    nc.vector.tensor_add(out=vt[:, mt, :], in0=ps, in1=bvb_t)

                # x is consumed only by the final residual add from here on:
                # fold the output-projection bias in now, on the idle ACT, so
                # the tail combine is two DVE ops instead of three.
                for pt in range(4):
                    nc.scalar.activation(out=xt[:, pt, :], in_=xt[:, pt, :],
                                         func=AF.Identity,
                                         bias=vecs_t[:, pt, 4:5], scale=1.0)

                # ---- scores^T + exp ------------------------------------------
                # The denominator needs sum-over-partitions of all 8 exp
                # tiles. Instead of 16 ones-matmuls (each streams 512 rows on
                # the PE), DVE -- idle during this phase -- pre-reduces the 8
                # tiles to 2 partials, leaving only 4 ones-matmuls per batch.
                et = epool.tile([128, 8, HW], R, tag="e")
                etp = epool.tile([128, 2, HW], R, tag="ep")
                psd = [gpool.tile([1, 512], F, tag="gn", name=f"psd{b}_{nh}")
                       for nh in range(2)]
                for mt in range(8):
                    pp2 = [mpool.tile([128, 512], F, tag="mm",
                                      name=f"sc{b}_{mt}_{nh}") for nh in range(2)]
                    for ot in range(4):
                        for nh in range(2):
                            nc.tensor.matmul(
                                pp2[nh],
                                kt[:, ot, mt * 128 : (mt + 1) * 128],
                                qt[:, ot, nh * 512 : (nh + 1) * 512],
                                start=(ot == 0), stop=(ot == 3))
                    for nh in range(2):
                        nc.scalar.activation(
                            out=et[:, mt, nh * 512 : (nh + 1) * 512], in_=pp2[nh],
                            func=AF.Exp, scale=float(C ** -0.5))
                    g = mt // 4
                    if mt % 4 == 1:
                        nc.vector.tensor_add(out=etp[:, g, :], in0=et[:, mt - 1, :],
                                             in1=et[:, mt, :])
                    elif mt % 4 >= 2:
                        nc.vector.tensor_add(out=etp[:, g, :], in0=etp[:, g, :],
                                             in1=et[:, mt, :])
                # ---- context + softmax denominator --------------------------
                # The denominator/reciprocal chain is emitted after the first
                # ctx accumulation group (which doesn't need it) so the PE
                # works through ctx matmuls instead of head-of-line blocking
                # on the DVE exp-reduction tail.
                rc = rpool.tile([1, HW], R, tag="recip")
                rb_sb = rpool.tile([128, 2, 512], F, tag="rb")
                ct_t = hpool.tile([128, 4, HW], R, tag="hctx")
                for c2 in range(4):
                    pp2 = [mpool.tile([128, 512], F, tag="mm",
                                      name=f"cx{b}_{c2}_{nh}") for nh in range(2)]
                    for mt in range(8):
                        for nh in range(2):
                            nc.tensor.matmul(
                                pp2[nh],
                                vt[:, mt, c2 * 128 : (c2 + 1) * 128],
                                et[:, mt, nh * 512 : (nh + 1) * 512],
                                start=(mt == 0), stop=(mt == 7))
                    if c2 == 0:
                        for nh in range(2):
                            for g in range(2):
                                nc.tensor.matmul(
                                    psd[nh], ones_col_t,
                                    etp[:, g, nh * 512 : (nh + 1) * 512],
                                    start=(g == 0), stop=(g == 1))
                        # broadcast first, then reciprocal on all 128
                        # partitions (a [1,512] reciprocal is serial on one
                        # partition and ~6x slower than the [128,512] one).
                        for nh in range(2):
                            nc.scalar.copy(out=rc[:, nh * 512 : (nh + 1) * 512],
                                           in_=psd[nh])
                            prb = gpool.tile([128, 512], F, tag="gn")
                            nc.tensor.matmul(prb, ones_row_t,
                                             rc[0:1, nh * 512 : (nh + 1) * 512],
                                             start=True, stop=True)
                            # denominators are in [~2e2, ~6e3]: far from the
                            # approx's undefined edge cases, and its ~2e-6 rel
                            # err is below the fp32r matmul noise floor.
                            nc.vector.reciprocal_approx_fast(
                                out=rb_sb[:, nh, :], in_=prb)
                    for nh in range(2):
                        # evacuate with the deferred softmax normalization
                        # folded in (ctx columns scaled by 1/den); the output
                        # projection is linear, so the final combine then needs
                        # only the residual add.
                        nc.vector.tensor_mul(
                            out=ct_t[:, c2, nh * 512 : (nh + 1) * 512],
                            in0=pp2[nh], in1=rb_sb[:, nh, :])

                # ---- output projection + residual ---------------------------
                for pt in range(4):
                    pp2 = [mpool.tile([128, 512], F, tag="mm",
                                      name=f"yp{b}_{pt}_{nh}") for nh in range(2)]
                    for c2 in range(4):
                        for nh in range(2):
                            nc.tensor.matmul(
                                pp2[nh],
                                wp_t[:, c2, pt * 128 : (pt + 1) * 128],
                                ct_t[:, c2, nh * 512 : (nh + 1) * 512],
                                start=(c2 == 0), stop=(c2 == 3))
                    for nh in range(2):
                        o_t = opool.tile([128, 512], F, tag="o1")
                        nc.vector.tensor_add(out=o_t, in0=pp2[nh],
                                             in1=xt[:, pt, nh * 512 : (nh + 1) * 512])
                        nc.sync.dma_start(
                            out=y.ap()[b][pt * 128 : (pt + 1) * 128, nh * 512 : (nh + 1) * 512],
                            in_=o_t)

    nc.finalize()
    return nc


def _get_nc():
    if "nc" not in _CACHE:
        _CACHE["nc"] = _build_nc()
    return _CACHE["nc"]


def make_in_maps(inputs):
    x = np.asarray(inputs["x"], np.float32).reshape(B, C, HW)
    f32 = lambda a: np.ascontiguousarray(np.asarray(a, np.float32))
    wqT = f32(inputs["wq"]).T.copy()
    wkT = f32(inputs["wk"]).T.copy()
    wvT = f32(inputs["wv"]).T.copy()
    wpT = f32(inputs["wp"]).T.copy()
    vstack = np.stack([f32(inputs["gn_w"]), f32(inputs["gn_b"]), f32(inputs["bq"]),
                       f32(inputs["bk"]), f32(inputs["bp"])])  # [5, C]
    # vecs[p, i, v] = vstack[v, i*128 + p]
    vecs = np.ascontiguousarray(vstack.reshape(5, 4, 128).transpose(2, 1, 0))
    bvb = np.broadcast_to(f32(inputs["bv"]), (128, 512)).copy()
    gmask = np.zeros((128, 8), np.float32)
    for p in range(128):
        gmask[p, p // GSIZE] = 1.0
    gmaskT = gmask.T.copy()
    ones_col = np.ones((128, 1), np.float32)
    ones_row = np.ones((1, 128), np.float32)

    shared = {"wq": wqT, "wk": wkT, "wv": wvT, "wp": wpT, "vecs": vecs, "bvb": bvb,
              "gmask": gmask, "gmaskT": gmaskT, "ones_col": ones_col, "ones_row": ones_row}
    return [dict(shared, x=np.ascontiguousarray(x[i * BPC : (i + 1) * BPC]))
            for i in range(NCORES)]


def kernel(**inputs) -> np.ndarray:
    from concourse.bass_utils import run_bass_kernel_spmd

    core_ids = list(range(NCORES))
    in_maps = make_in_maps(inputs)
    nc = _get_nc()
    res = run_bass_kernel_spmd(nc, in_maps, core_ids)
    out = np.concatenate([res.results[i]["y"] for i in core_ids], axis=0)
    return out.reshape(B, C, H, W)



# revision 5
# speedup vs baseline: 1.5763x; 1.5763x over previous
"""AttnBlock (GroupNorm + single-head self-attention + residual) on 8 TRN2 cores.

Strategy: data-parallel over batch (16 images -> 2 per core), no collectives.

Key speedups over a pure-fp32r formulation:
  * The q/k projections are folded on the host: s = q.k = h (Wq^T Wk) h^T, so
    the device computes g = M^T h once (M = Wq^T Wk precomputed in f64) and
    scores directly from (g, h) -- one of the four C*C projections disappears.
    The bq bias becomes a per-channel additive term on g (exact, folded into
    the g evacuation); the bk bias shifts every score in a row n equally and
    cancels in softmax, so it is dropped.
  * The N^2-sized matmuls (scores, context) plus the v/out projections run in
    fp8e4 with DoubleRow perf mode: two 128-deep contraction tiles per pass =
    2x PE throughput. Softmax averaging dilutes the quantization noise; the
    g-projection (whose error would be amplified through exp) stays fp32r.
  * exp is computed as exp(s/sqrt(C) - 4): keeps e under fp8e4's 240 max
    (scores reach ~6.7) and the shift cancels through the softmax denominator,
    which is accumulated from the same shifted fp8 values by tiny ones-matmuls.
  * Softmax normalization is deferred through the linear output projection
    (ctx columns scaled by 1/den at evacuation), so the tail is one fused
    (psum + bp) + x op per tile.

Per-batch dataflow on one core (C=512 channels, N=H*W=1024 tokens):
  x    [C, N]  channel-major
  h    = groupnorm(x)          (stats via bn_stats + tiny matmuls; h in f32r,
                                plus an fp8 copy h8 quantized on the Pool engine)
  g8   [d, n] = (M^T h + u)    (fp32r matmuls; u = Wk^T bq, zero in practice)
  v8   [m, c] = h8^T Wv8 + bv  (fp8 DoubleRow, token-major)
  sT   [m, n] = h8^T g8        (fp8 DoubleRow) -> e8 = exp(sT/sqrt(C) - 4)
  den  [1, n] = ones8^T e8     (fp8 DoubleRow ones-matmuls, PSUM-accumulated)
  ctx  [c, n] = v8^T e8        (fp8 DoubleRow) * bcast(1/den) -> ct8 (fp8)
  y    [p, n] = Wp8^T ct8      (fp8 DoubleRow)
  out  = (y + bp) + x
"""

import numpy as np

B, C, HW = 16, 512, 1024
H = W = 32
NCORES = 8
BPC = B // NCORES
GROUPS = 32
GSIZE = C // GROUPS  # 16
EPS = 1e-5
SHIFT = 4.0  # exp(s - SHIFT); cancels via the denominator

_CACHE = {}


def _build_nc():
    import concourse.bacc as bacc
    import concourse.tile as tile
    from concourse import mybir

    R = mybir.dt.float32r
    F = mybir.dt.float32
    F8 = mybir.dt.float8e4
    A = mybir.AluOpType
    AF = mybir.ActivationFunctionType
    DR = mybir.MatmulPerfMode.DoubleRow

    nc = bacc.Bacc("TRN2", target_bir_lowering=False, debug=False)

    x = nc.declare_dram_parameter("x", [BPC, C, HW], F, isOutput=False)
    wm = nc.declare_dram_parameter("wm", [C, C], R, isOutput=False)  # M = wq^T wk
    wv8 = nc.declare_dram_parameter("wv8", [C, C], F8, isOutput=False)
    wp8 = nc.declare_dram_parameter("wp8", [C, C], F8, isOutput=False)
    vecs = nc.declare_dram_parameter("vecs", [128, 4, 4], F, isOutput=False)
    bvb = nc.declare_dram_parameter("bvb", [128, 512], F, isOutput=False)
    gmask = nc.declare_dram_parameter("gmask", [128, 8], F, isOutput=False)
    gmaskT = nc.declare_dram_parameter("gmaskT", [8, 128], F, isOutput=False)
    ones8 = nc.declare_dram_parameter("ones8", [128, 2, 16], F8, isOutput=False)
    ones_row = nc.declare_dram_parameter("ones_row", [1, 128], R, isOutput=False)
    y = nc.declare_dram_parameter("y", [BPC, C, HW], F, isOutput=True)

    with tile.TileContext(nc) as tc:
        import contextlib

        ctx = contextlib.ExitStack()
        with ctx:
            wpool = ctx.enter_context(tc.tile_pool(name="w", bufs=1))
            cpool = ctx.enter_context(tc.tile_pool(name="c", bufs=1))
            xpool = ctx.enter_context(tc.tile_pool(name="x", bufs=2))
            hpool = ctx.enter_context(tc.tile_pool(name="h", bufs=2))
            h8pool = ctx.enter_context(tc.tile_pool(name="h8", bufs=2))
            qpool = ctx.enter_context(tc.tile_pool(name="q", bufs=2))
            vpool = ctx.enter_context(tc.tile_pool(name="v", bufs=2))
            epool = ctx.enter_context(tc.tile_pool(name="e", bufs=2))
            spool = ctx.enter_context(tc.tile_pool(name="s", bufs=2))
            rpool = ctx.enter_context(tc.tile_pool(name="r", bufs=2))
            opool = ctx.enter_context(tc.tile_pool(name="o", bufs=3))
            mpool = ctx.enter_context(tc.tile_pool(name="mp", bufs=6, space="PSUM"))
            gpool = ctx.enter_context(tc.tile_pool(name="gp", bufs=2, space="PSUM"))

            # ---- persistent loads -------------------------------------------
            xts = []
            for b in range(BPC):
                xt_b = xpool.tile([128, 4, HW], F, tag="x", name=f"xt{b}")
                xts.append(xt_b)
            xsrc = [x.ap()[b].rearrange("(i p) n -> p i n", p=128) for b in range(BPC)]
            from concourse.tile import add_dep_helper

            # DMA order = HBM-bandwidth priority order (batch-0 x gates
            # groupnorm stats and the whole pipeline).
            x0_dmas = []
            for i in range(4):
                for s in range(2):
                    d = nc.sync.dma_start(out=xts[0][:, i, s * 512 : (s + 1) * 512],
                                          in_=xsrc[0][:, i, s * 512 : (s + 1) * 512])
                    x0_dmas.append(d)
            gmask_t = cpool.tile([128, 8], F, tag="gmask")
            nc.sync.dma_start(out=gmask_t, in_=gmask.ap())
            gmaskT_t = cpool.tile([8, 128], F, tag="gmaskT")
            nc.sync.dma_start(out=gmaskT_t, in_=gmaskT.ap())
            vecs_t = cpool.tile([128, 4, 4], F, tag="vecs")
            nc.sync.dma_start(out=vecs_t, in_=vecs.ap())
            bvb_t = cpool.tile([128, 512], F, tag="bvb")
            nc.sync.dma_start(out=bvb_t, in_=bvb.ap())
            # pair-dim stride must be a multiple of 16 for DoubleRow ldweights
            ones8_t = cpool.tile([128, 2, 16], F8, tag="ones8")
            nc.sync.dma_start(out=ones8_t, in_=ones8.ap())
            ones_row_t = cpool.tile([1, 128], R, tag="ones_row")
            nc.sync.dma_start(out=ones_row_t, in_=ones_row.ap())
            eps8 = cpool.tile([8, 1], F, tag="eps8")
            nc.vector.memset(eps8, EPS)
            nshift = cpool.tile([128, 1], F, tag="nshift")
            nc.vector.memset(nshift, -SHIFT)

            # PE warmup: keeps the tensor engine busy (and the clock ramped)
            # while batch-0 x and the groupnorm stats crunch through.
            wrm = cpool.tile([128, 128], F, tag="wrm")
            nc.vector.memset(wrm, 0.0)
            wps = mpool.tile([128, 512], F, tag="mm")

            def warmup(n):
                for j in range(n):
                    nc.tensor.matmul(wps[:, 0:128], wrm, wrm, start=(j == 0),
                                     stop=(j == n - 1))

            warmup(24)

            wm_t = wpool.tile([128, 4, C], R, tag="wm")
            wv8_t = wpool.tile([128, 4, C], F8, tag="wv8")
            wp8_t = wpool.tile([128, 4, C], F8, tag="wp8")
            prev = x0_dmas[-1]
            bulk = [(wm_t, wm, None), (wv8_t, wv8, None), (None, None, 1),
                    (wp8_t, wp8, None)]
            for t, src, xb in bulk:
                if xb is not None:
                    for i in range(4):
                        d = nc.sync.dma_start(out=xts[xb][:, i, :], in_=xsrc[xb][:, i, :])
                        add_dep_helper(d.ins, prev.ins, reason="dma bandwidth order")
                    prev = d
                else:
                    d = nc.sync.dma_start(
                        out=t, in_=src.ap().rearrange("(ct p) o -> p ct o", p=128))
                    add_dep_helper(d.ins, prev.ins, reason="dma bandwidth order")
                    prev = d

            # ---- groupnorm for both batches, pipelined per 128-channel tile.
            # Identical structure to the fp32r kernel; additionally each
            # normalized tile is re-quantized to fp8 on the Pool engine (the
            # only engine with cycles to spare here), producing h8 for the
            # DoubleRow matmuls.
            hts = []
            ht8s = []
            for b in range(BPC):
                xt = xts[b]
                ht = hpool.tile([128, 4, HW], R, tag="hctx", name=f"ht{b}")
                hts.append(ht)
                ht8 = h8pool.tile([128, 4, HW], F8, tag="h8", name=f"ht8_{b}")
                ht8s.append(ht8)
                varga = spool.tile([8, 4], F, tag="varga")
                sda = spool.tile([8, 4], F, tag="sda")
                ggs = {}

                def finish(i, gg, b=b, xt=xt, ht=ht, ht8=ht8, sda=sda):
                    st2 = spool.tile([8, 2], F, tag=f"st2{i}")
                    with nc.allow_low_precision("groupnorm rstd"):
                        nc.vector.reciprocal(out=st2[:, 0:1], in_=sda[:, i : i + 1])
                    nc.vector.tensor_copy(out=st2[:, 1:2], in_=gg[:, 0:1])
                    bc = gpool.tile([128, 2], F, tag="gn")
                    nc.tensor.matmul(bc, gmaskT_t, st2, start=True, stop=True)
                    scale_c = spool.tile([128, 1], F, tag=f"scale{i}")
                    nc.vector.tensor_mul(out=scale_c, in0=bc[:, 0:1], in1=vecs_t[:, i, 0:1])
                    tmp = spool.tile([128, 1], F, tag=f"tmp{i}")
                    nc.vector.tensor_mul(out=tmp, in0=bc[:, 1:2], in1=scale_c)
                    shift_c = spool.tile([128, 1], F, tag=f"shift{i}")
                    nc.vector.tensor_sub(out=shift_c, in0=vecs_t[:, i, 1:2], in1=tmp)
                    if b == 0 and i < 3:
                        warmup(8 + 2 * i)
                    if b == 0:
                        nc.scalar.activation(out=ht[:, i, :], in_=xt[:, i, :],
                                             func=AF.Identity, bias=shift_c,
                                             scale=scale_c)
                    else:
                        nc.vector.tensor_scalar(
                            out=ht[:, i, :], in0=xt[:, i, :],
                            scalar1=scale_c, scalar2=shift_c, op0=A.mult, op1=A.add)
                    # fp8 shadow copy for the DoubleRow operands (Pool engine,
                    # SBUF->SBUF; Pool cannot touch PSUM so this is its niche)
                    nc.gpsimd.tensor_scalar(
                        out=ht8[:, i, :], in0=xt[:, i, :],
                        scalar1=scale_c, scalar2=shift_c, op0=A.mult, op1=A.add)

                for i in range(4):
                    xr = xt[:, i, :].rearrange("p (s d) -> p s d", d=512)
                    st6 = spool.tile([128, 2, 6], F, tag=f"st6{i}")
                    for s in range(2):
                        nc.vector.bn_stats(out=st6[:, s, :], in_=xr[:, s, :])
                    mv = spool.tile([128, 2], F, tag=f"mv{i}")
                    nc.vector.bn_aggr(out=mv, in_=st6)
                    stats_i = spool.tile([128, 2], F, tag=f"stats{i}")
                    m2c = spool.tile([128, 1], F, tag=f"m2c{i}")
                    nc.vector.tensor_mul(out=m2c, in0=mv[:, 0:1], in1=mv[:, 0:1])
                    nc.vector.tensor_add(out=stats_i[:, 1:2], in0=mv[:, 1:2], in1=m2c)
                    nc.vector.tensor_copy(out=stats_i[:, 0:1], in_=mv[:, 0:1])
                    gps = gpool.tile([8, 2], F, tag="gn")
                    nc.tensor.matmul(gps, gmask_t, stats_i, start=True, stop=True)
                    gg = spool.tile([8, 2], F, tag=f"gg{i}")
                    ggs[i] = gg
                    nc.vector.tensor_scalar_mul(out=gg, in0=gps, scalar1=1.0 / GSIZE)
                    m2g = spool.tile([8, 1], F, tag=f"m2g{i}")
                    nc.vector.tensor_mul(out=m2g, in0=gg[:, 0:1], in1=gg[:, 0:1])
                    nc.vector.tensor_sub(out=varga[:, i : i + 1], in0=gg[:, 1:2],
                                         in1=m2g)
                    if b == 0:
                        nc.scalar.activation(out=sda[:, i : i + 1],
                                             in_=varga[:, i : i + 1],
                                             func=AF.Sqrt, bias=eps8, scale=1.0)
                        finish(i, gg)
                if b == 1:
                    nc.scalar.activation(out=sda, in_=varga, func=AF.Sqrt,
                                         bias=eps8, scale=1.0)
                    for i in range(4):
                        finish(i, ggs[i])

            for b in range(BPC):
                xt = xts[b]
                ht = hts[b]
                ht8 = ht8s[b]
                # ---- g-projection (fp32r): g = M^T h (+u per-channel) -------
                gt8 = qpool.tile([128, 4, HW], F8, tag="g8")
                for ot in range(4):
                    pp2 = [mpool.tile([128, 512], F, tag="mm",
                                      name=f"pj{b}_{ot}_{nh}") for nh in range(2)]
                    for ct in range(4):
                        for nh in range(2):
                            nc.tensor.matmul(
                                pp2[nh],
                                wm_t[:, ct, ot * 128 : (ot + 1) * 128],
                                ht[:, ct, nh * 512 : (nh + 1) * 512],
                                start=(ct == 0), stop=(ct == 3))
                    for nh in range(2):
                        # u-bias (Wk^T bq) folded in; fp8 out for DoubleRow
                        nc.scalar.activation(
                            out=gt8[:, ot, nh * 512 : (nh + 1) * 512],
                            in_=pp2[nh], func=AF.Identity,
                            bias=vecs_t[:, ot, 3:4], scale=1.0)

                # ---- v-projection (fp8 DoubleRow, token-major) --------------
                vt8 = vpool.tile([128, 8, 512], F8, tag="v8")
                for mt in range(8):
                    ps = mpool.tile([128, 512], F, tag="mm")
                    for g in range(2):
                        nc.tensor.matmul(
                            ps,
                            ht8[:, 2 * g : 2 * g + 2, mt * 128 : (mt + 1) * 128],
                            wv8_t[:, 2 * g : 2 * g + 2, :],
                            start=(g == 0), stop=(g == 1), perf_mode=DR)
                    nc.vector.tensor_add(out=vt8[:, mt, :], in0=ps, in1=bvb_t)

                # ---- scores^T + exp (fp8 DoubleRow) -------------------------
                et8 = epool.tile([128, 8, HW], F8, tag="e8")
                for mt in range(8):
                    pp2 = [mpool.tile([128, 512], F, tag="mm",
                                      name=f"sc{b}_{mt}_{nh}") for nh in range(2)]
                    for g in range(2):
                        for nh in range(2):
                            nc.tensor.matmul(
                                pp2[nh],
                                ht8[:, 2 * g : 2 * g + 2, mt * 128 : (mt + 1) * 128],
                                gt8[:, 2 * g : 2 * g + 2, nh * 512 : (nh + 1) * 512],
                                start=(g == 0), stop=(g == 1), perf_mode=DR)
                    for nh in range(2):
                        nc.scalar.activation(
                            out=et8[:, mt, nh * 512 : (nh + 1) * 512], in_=pp2[nh],
                            func=AF.Exp, scale=float(C ** -0.5), bias=nshift)

                # ---- context (fp8 DoubleRow), with the denominator chain ----
                # emitted after the first ctx accumulation group so the PE is
                # never head-of-line blocked on the exp tail.
                rc = rpool.tile([1, HW], R, tag="recip")
                rb_sb = rpool.tile([128, 2, 512], F, tag="rb")
                ct8 = h8pool.tile([128, 4, HW], F8, tag="h8", name=f"ct8_{b}")
                psd = [gpool.tile([1, 512], F, tag="gn", name=f"psd{b}_{nh}")
                       for nh in range(2)]
                cps = {}
                for c2 in range(4):
                    pp2 = [mpool.tile([128, 512], F, tag="mm",
                                      name=f"cx{b}_{c2}_{nh}") for nh in range(2)]
                    cps[c2] = pp2
                    for g in range(4):
                        for nh in range(2):
                            nc.tensor.matmul(
                                pp2[nh],
                                vt8[:, 2 * g : 2 * g + 2, c2 * 128 : (c2 + 1) * 128],
                                et8[:, 2 * g : 2 * g + 2, nh * 512 : (nh + 1) * 512],
                                start=(g == 0), stop=(g == 3), perf_mode=DR)
                    if c2 == 0:
                        # softmax denominator: fp8 ones-matmuls over the same
                        # shifted e8 tiles (errors cancel in the ratio)
                        for nh in range(2):
                            for g in range(4):
                                nc.tensor.matmul(
                                    psd[nh], ones8_t[:, :, 0:1],
                                    et8[:, 2 * g : 2 * g + 2, nh * 512 : (nh + 1) * 512],
                                    start=(g == 0), stop=(g == 3), perf_mode=DR)
                        for nh in range(2):
                            nc.scalar.copy(out=rc[:, nh * 512 : (nh + 1) * 512],
                                           in_=psd[nh])
                            prb = gpool.tile([128, 512], F, tag="gn")
                            nc.tensor.matmul(prb, ones_row_t,
                                             rc[0:1, nh * 512 : (nh + 1) * 512],
                                             start=True, stop=True)
                            nc.vector.reciprocal_approx_fast(
                                out=rb_sb[:, nh, :], in_=prb)
                    for nh in range(2):
                        nc.vector.tensor_mul(
                            out=ct8[:, c2, nh * 512 : (nh + 1) * 512],
                            in0=pp2[nh], in1=rb_sb[:, nh, :])

                # ---- output projection (fp8 DoubleRow) + bias + residual ----
                for pt in range(4):
                    pp2 = [mpool.tile([128, 512], F, tag="mm",
                                      name=f"yp{b}_{pt}_{nh}") for nh in range(2)]
                    for g in range(2):
                        for nh in range(2):
                            nc.tensor.matmul(
                                pp2[nh],
                                wp8_t[:, 2 * g : 2 * g + 2, pt * 128 : (pt + 1) * 128],
                                ct8[:, 2 * g : 2 * g + 2, nh * 512 : (nh + 1) * 512],
                                start=(g == 0), stop=(g == 1), perf_mode=DR)
                    for nh in range(2):
                        o_t = opool.tile([128, 512], F, tag="o1")
                        nc.vector.scalar_tensor_tensor(
                            out=o_t, in0=pp2[nh], scalar=vecs_t[:, pt, 2:3],
                            in1=xt[:, pt, nh * 512 : (nh + 1) * 512],
                            op0=A.add, op1=A.add)
                        nc.sync.dma_start(
                            out=y.ap()[b][pt * 128 : (pt + 1) * 128, nh * 512 : (nh + 1) * 512],
                            in_=o_t)

    nc.finalize()
    return nc


def _get_nc():
    if "nc" not in _CACHE:
        _CACHE["nc"] = _build_nc()
    return _CACHE["nc"]


def make_in_maps(inputs):
    import ml_dtypes

    E4 = ml_dtypes.float8_e4m3
    x = np.asarray(inputs["x"], np.float32).reshape(B, C, HW)
    f32 = lambda a: np.ascontiguousarray(np.asarray(a, np.float32))
    f64 = lambda a: np.asarray(a, np.float64)
    # M = wq^T wk so that s_nm = h_n^T M h_m  (folds the q/k projections)
    M = (f64(inputs["wq"]).T @ f64(inputs["wk"])).astype(np.float32)
    # u = wk^T bq: the only bq term that survives softmax; added onto g
    u = (f64(inputs["wk"]).T @ f64(inputs["bq"])).astype(np.float32)
    wvT8 = f32(inputs["wv"]).T.copy().astype(E4)
    wpT8 = f32(inputs["wp"]).T.copy().astype(E4)
    vstack = np.stack([f32(inputs["gn_w"]), f32(inputs["gn_b"]),
                       f32(inputs["bp"]), u])  # [4, C]
    vecs = np.ascontiguousarray(vstack.reshape(4, 4, 128).transpose(2, 1, 0))
    bvb = np.broadcast_to(f32(inputs["bv"]), (128, 512)).copy()
    gmask = np.zeros((128, 8), np.float32)
    for p in range(128):
        gmask[p, p // GSIZE] = 1.0
    gmaskT = gmask.T.copy()
    ones8 = np.ones((128, 2, 16), np.float32).astype(E4)
    ones_row = np.ones((1, 128), np.float32)

    shared = {"wm": M, "wv8": wvT8, "wp8": wpT8, "vecs": vecs, "bvb": bvb,
              "gmask": gmask, "gmaskT": gmaskT, "ones8": ones8,
              "ones_row": ones_row}
    return [dict(shared, x=np.ascontiguousarray(x[i * BPC : (i + 1) * BPC]))
            for i in range(NCORES)]


def kernel(**inputs) -> np.ndarray:
    from concourse.bass_utils import run_bass_kernel_spmd

    core_ids = list(range(NCORES))
    in_maps = make_in_maps(inputs)
    nc = _get_nc()
    res = run_bass_kernel_spmd(nc, in_maps, core_ids)
    out = np.concatenate([res.results[i]["y"] for i in core_ids], axis=0)
    return out.reshape(B, C, H, W)
